# revision 2
# baseline (speedup 1.0000x reference)
"""Causal GQA attention (S=2048, B=2, HQ=32, HKV=8, D=128) on 8 trn2 cores.

Sharding: the 16 (batch, kv-head) pairs are split 2 per core (data+head
parallel); each pair carries group=4 query heads -> 8 attention heads/core.

v2 dataflow (vs the per-k-tile-ACTIVATE baseline): QK^T staging tiles are
packed back-to-back into a 5-bank PSUM ring (2560 fp32 cols).  The ring is
consumed by WIDE exp ACTIVATEs over bank-aligned windows that alternate
1536/1024 cols, cutting ScalarE per-instruction overhead ~2.6x.  Each
(head, q-chunk, k-tile) region lands at an arbitrary 128-aligned ring
offset; its QK matmuls split at PSUM bank boundaries and its consumers
(AV matmul, DVE row-sum accumulation) slice the window's SBUF exp tile.

The causal mask moved from the DVE to the PE: the diagonal 128x128 block
gets a second accumulating matmul diag(-2000) @ strict_lower_tri, driving
masked logits to -2000 so exp(SCALE*x) underflows to exactly 0.

Softmax denominators are pure-DVE: bf16 element-wise accumulation of each
window slice into a per-chunk sacc tile, folded at chunk end by two
ones-column matmuls into a dedicated 1-bank PSUM row pair (parity-
alternated), then srow -> DMA-reshape [128,8] -> reciprocal -> DRAM ->
partition-broadcast -> multiply -> store, advanced one stage per region
(pipelined across chunks).

PSUM budget: ring 5 banks + sum rows 1 bank + out^T accumulator 2 banks
= 8 exactly.  Host side only re-lays-out data (and casts to bf16).
"""

import numpy as np
import ml_dtypes

import concourse.bass as bass
import concourse.mybir as mybir
import concourse.tile as tile
from concourse import bacc, bass_utils
from concourse.masks import make_lower_triangular

S, B, HQ, HKV, D = 2048, 2, 32, 8, 128
G = HQ // HKV                      # 4 query heads per kv head
NCORES = 8
NPAIRS = B * HKV                   # 16 (batch, kv-head) pairs
PAIRS_PER_CORE = NPAIRS // NCORES  # 2
HEADS_PER_CORE = PAIRS_PER_CORE * G  # 8
SCALE = 1.0 / float(np.sqrt(D))
CH = 1024                          # q-chunk width
NCH = S // CH                      # 2
KT = 128                           # k-tile width
NKT = S // KT                      # 16
RING = 2560                        # 5 PSUM banks of staged logits
NEGC = -2000.0                     # causal mask additive constant

F32 = mybir.dt.float32
BF16 = mybir.dt.bfloat16
NP_BF16 = ml_dtypes.bfloat16


def _win_of(g):
    """Global fill coord -> window id (windows alternate 1536/1024 wide)."""
    cyc, r = divmod(g, RING)
    return 2 * cyc + (0 if r < 1536 else 1)


def _win_start(w):
    return RING * (w // 2) + (0 if w % 2 == 0 else 1536)


def _win_base(w):
    return 0 if w % 2 == 0 else 1536


def _win_width(w):
    return 1536 if w % 2 == 0 else 1024


def emit_core_program(tc, qt, kt, v, recd, ot):
    from contextlib import ExitStack

    nc = tc.nc
    with ExitStack() as ctx:
        _emit_core_program(ctx, tc, nc, qt, kt, v, recd, ot)


def _emit_core_program(ctx, tc, nc, qt, kt, v, recd, ot):
    singles = ctx.enter_context(tc.tile_pool(name="singles", bufs=1))
    kv_pool = ctx.enter_context(tc.tile_pool(name="kv", bufs=2))
    q_pool = ctx.enter_context(tc.tile_pool(name="q", bufs=2))
    pt_pool = ctx.enter_context(tc.tile_pool(name="pt", bufs=3))
    sacc_pool = ctx.enter_context(tc.tile_pool(name="sacc", bufs=2))
    osb_pool = ctx.enter_context(tc.tile_pool(name="osb", bufs=3))
    osb2_pool = ctx.enter_context(tc.tile_pool(name="osb2", bufs=3))
    bcs_pool = ctx.enter_context(tc.tile_pool(name="bcs", bufs=3))
    srow_pool = ctx.enter_context(tc.tile_pool(name="srow", bufs=3))
    srec_pool = ctx.enter_context(tc.tile_pool(name="srec", bufs=3))
    ring_pool = ctx.enter_context(tc.tile_pool(name="ring", bufs=1, space="PSUM"))
    oa_pool = ctx.enter_context(tc.tile_pool(name="oa", bufs=1, space="PSUM"))
    ps_sum = ctx.enter_context(tc.tile_pool(name="ps_sum", bufs=1, space="PSUM"))

    # Constants.
    trislf = singles.tile([128, 128], F32)
    make_lower_triangular(nc, trislf[:], val=1.0, diag=False)  # 1 where p > j
    trisl = singles.tile([128, 128], BF16)
    nc.scalar.copy(out=trisl[:], in_=trislf[:])
    negcf = singles.tile([128, 128], F32)
    nc.gpsimd.memset(negcf[:], NEGC)
    nc.gpsimd.affine_select(            # keep NEGC on the diagonal, 0 off it
        out=negcf[:], in_=negcf[:],
        compare_op=mybir.AluOpType.is_equal, fill=0.0,
        base=0, pattern=[[-1, 128]], channel_multiplier=1,
    )
    negc = singles.tile([128, 128], BF16)
    nc.scalar.copy(out=negc[:], in_=negcf[:])
    onesc = singles.tile([128, 1], BF16)   # ones column (sum-over-k lhsT)
    nc.vector.memset(onesc[:], 1.0)

    ring = ring_pool.tile([128, RING], F32)
    sum_ps = ps_sum.tile([128, 512], F32)
    # srow pulls [33, 512] windows from this bank; the 31 partitions between
    # the two fold rows are never written -- zero once so reads are defined
    nc.vector.memset(sum_ps[:], 0.0)

    kv_tiles = {}

    def ensure_pair(pair):
        if pair in kv_tiles or pair >= PAIRS_PER_CORE:
            return
        kt_sb = kv_pool.tile([D, S], BF16, tag="kt", name=f"kt_{pair}")
        nc.sync.dma_start(out=kt_sb[:], in_=kt[pair])
        v_sb = kv_pool.tile([128, NKT * D], BF16, tag="v", name=f"v_{pair}")
        nc.sync.dma_start(out=v_sb[:], in_=v[pair])
        kv_tiles[pair] = (kt_sb, v_sb)

    q_tiles = {}

    def ensure_head(head):
        if head in q_tiles or head >= HEADS_PER_CORE:
            return
        q_sb = q_pool.tile([D, S], BF16, tag="q", name=f"q_{head}")
        nc.sync.dma_start(out=q_sb[:], in_=qt[head])
        q_tiles[head] = q_sb

    # Flat schedule of regions: (head, chunk, kti)
    sched = []
    for head in range(HEADS_PER_CORE):
        for c in range(NCH):
            for kti in range(8 * c + 8):
                sched.append((head, c, kti))

    # --- mutable emission state ---
    state = dict(gpos=0, win_id=0, sealed=-1)
    win_tiles = {}     # win id -> (pt tile, global window start)
    region_meta = []   # per region: (ga, qa, width)
    consumer_q = []    # (last_win_needed, region index)
    oacc = {}          # (head, c) -> psum accumulator tile
    av_started = {}    # (head, c) -> set of banks already started
    saccs = {}         # (head, c) -> sacc tile
    pending = []       # chunk-tail normalization pipeline entries
    tail_count = [0]

    def seal_window(w, partial_end=None):
        gs = _win_start(w)
        width = (partial_end - gs) if partial_end is not None else _win_width(w)
        base = _win_base(w)
        ptw = pt_pool.tile([128, 1536], BF16, tag="pt", name=f"pt_{w}")
        nc.scalar.activation(
            ptw[:, 0:width], ring[:, base:base + width],
            mybir.ActivationFunctionType.Exp, scale=SCALE)
        win_tiles[w] = (ptw, gs)
        state['sealed'] = w
        # flush all consumers whose last window is now sealed
        while consumer_q and consumer_q[0][0] <= state['sealed']:
            _, ridx = consumer_q.pop(0)
            emit_consumers(ridx)

    def maybe_seal():
        while state['gpos'] >= _win_start(state['win_id']) + _win_width(state['win_id']):
            seal_window(state['win_id'])
            state['win_id'] += 1

    def emit_qk(i):
        head, c, kti = sched[i]
        if c == 0 and kti == 0:
            pair = head // G
            ensure_pair(pair + 1)
            ensure_head(head + 1)
        kt_sb, _ = kv_tiles[head // G]
        q_sb = q_tiles[head]
        off = max(0, 128 * kti - CH * c)
        w = CH - off
        has_diag = 128 * kti >= CH * c
        ga = state['gpos']
        region_meta.append((ga, off, w))
        # spans: split [ga, ga+w) at RING wrap
        spans = []
        g0 = ga
        rem = w
        while rem > 0:
            ra = g0 % RING
            ln = min(rem, RING - ra)
            spans.append((g0, ra, ln))
            g0 += ln
            rem -= ln
        first = True
        for (gsp, ra, ln) in spans:
            # pieces: split at PSUM bank boundaries (512 multiples of ring coord)
            p = 0
            while p < ln:
                seg = min(ln - p, 512 - (ra + p) % 512)
                a = ra + p
                qa = off + (gsp - ga) + p
                diag_here = has_diag and first and p == 0
                nc.tensor.matmul(
                    out=ring[:, a:a + seg],
                    lhsT=kt_sb[:, 128 * kti:128 * (kti + 1)],
                    rhs=q_sb[:, CH * c + qa:CH * c + qa + seg],
                    start=True, stop=(not diag_here),
                )
                if diag_here:
                    nc.tensor.matmul(
                        out=ring[:, a:a + 128],
                        lhsT=negc[:], rhs=trisl[:],
                        start=False, stop=True,
                    )
                p += seg
            first = False
        state['gpos'] += w
        # queue consumers keyed on the last window this region touches
        last_win = _win_of(state['gpos'] - 1)
        consumer_q.append((last_win, i))
        maybe_seal()

    def _pt_slice(g):
        """global ring coord -> (pt tile, rel offset, cols remaining in window)"""
        wd = _win_of(g)
        ptw, gs = win_tiles[wd]
        rel = g - gs
        return ptw, rel, _win_width(wd) - rel

    def emit_consumers(i):
        head, c, kti = sched[i]
        _, v_sb = kv_tiles[head // G]
        ga, qa, w = region_meta[i]
        qb = qa + w
        last = kti == 8 * c + 7
        key = (head, c)

        # --- denominators: sacc += exp slice (bf16, DVE) ---
        if kti == 0:
            sacc = sacc_pool.tile([128, CH], BF16, tag="sacc",
                                  name=f"sacc_{head}_{c}")
            saccs[key] = sacc
            first_sacc = True
        else:
            sacc = saccs[key]
            first_sacc = False
        q0 = qa
        while q0 < qb:
            ptw, rel, win_rem = _pt_slice(ga + (q0 - qa))
            q1 = min(qb, q0 + win_rem)
            if first_sacc:
                nc.vector.tensor_copy(sacc[:, q0:q1], ptw[:, rel:rel + (q1 - q0)])
            else:
                nc.vector.tensor_add(
                    sacc[:, q0:q1], sacc[:, q0:q1], ptw[:, rel:rel + (q1 - q0)])
            q0 = q1

        # --- out^T += V^T P^T ---
        if kti == 0:
            oacc[key] = oa_pool.tile([128, CH], F32, tag="oacc",
                                     name=f"oa_{head}_{c}")
            av_started[key] = set()
        oa = oacc[key]
        started = av_started[key]
        q0 = qa
        while q0 < qb:
            ptw, rel, win_rem = _pt_slice(ga + (q0 - qa))
            # split at q-bank boundary (512) and window end
            q1 = min(qb, q0 + win_rem, (q0 // 512 + 1) * 512)
            bank = q0 // 512
            stop_kti = 8 * c + 3 if bank == 0 else 8 * c + 7
            nc.tensor.matmul(
                out=oa[:, q0:q1],
                lhsT=v_sb[:, D * kti:D * (kti + 1)],
                rhs=ptw[:, rel:rel + (q1 - q0)],
                start=(bank not in started),
                stop=(kti == stop_kti and (q1 == qb or q1 % 512 == 0)),
            )
            started.add(bank)
            q0 = q1

        if last:
            # chunk tail: fold sacc into PSUM sum rows; evacuate oacc
            sacc = saccs.pop(key)
            rb = 64 * (tail_count[0] % 2)
            tail_count[0] += 1
            nc.tensor.matmul(
                out=sum_ps[rb:rb + 1, 0:512], lhsT=onesc[:],
                rhs=sacc[:, 0:512], start=True, stop=True,
                tile_position=(0, rb),
            )
            nc.tensor.matmul(
                out=sum_ps[rb + 32:rb + 33, 0:512], lhsT=onesc[:],
                rhs=sacc[:, 512:CH], start=True, stop=True,
                tile_position=(0, rb + 32),
            )
            oa = oacc.pop(key)
            av_started.pop(key)
            osb = osb_pool.tile([128, CH], BF16, tag="osb",
                                name=f"osb_{head}_{c}")
            nc.vector.tensor_copy(osb[:], oa[:])
            pending.append(dict(head=head, c=c, stage=0, osb=osb,
                                rb=rb, born=i))

    def advance_norm(drain=False):
        for ent in list(pending):
            head, c, st = ent["head"], ent["c"], ent["stage"]
            if st == 0:
                rb = ent["rb"]
                srow = srow_pool.tile([33, 512], F32, tag="srow",
                                      name=f"srow_{head}_{c}")
                nc.vector.tensor_copy(srow[:], sum_ps[rb:rb + 33, 0:512])
                srec = srec_pool.tile([128, NCH * 4], F32, tag="srec",
                                      name=f"srec_{head}_{c}")
                nc.sync.dma_start(out=srec[0:64, :], in_=srow[0:1, :])
                nc.sync.dma_start(out=srec[64:128, :], in_=srow[32:33, :])
                ent["srec"] = srec
            elif st == 1:
                srec2 = srec_pool.tile([128, NCH * 4], BF16, tag="srec2",
                                       name=f"srec2_{head}_{c}")
                with nc.allow_low_precision(reason="1/sum broadcast in bf16"):
                    nc.vector.reciprocal(out=srec2[:], in_=ent["srec"][:])
                nc.sync.dma_start(out=recd[head, c], in_=srec2[:])
            elif st == 2:
                bcs = bcs_pool.tile([128, CH], BF16, tag="bcs",
                                    name=f"bcs_{head}_{c}")
                nc.sync.dma_start(
                    out=bcs[:], in_=recd[head, c].partition_broadcast(128))
                ent["bcs"] = bcs
            elif st == 3:
                osb2 = osb2_pool.tile([128, CH], BF16, tag="osb2",
                                      name=f"osb2_{head}_{c}")
                nc.vector.tensor_mul(osb2[:], ent["osb"][:], ent["bcs"][:])
                nc.sync.dma_start(
                    out=ot[head][:, CH * c:CH * (c + 1)], in_=osb2[:])
                pending.remove(ent)
            ent["stage"] = st + 1
            break

    # --- main emission loop: QK runs 3 regions ahead of consumers ---
    ensure_pair(0)
    ensure_head(0)
    LOOKAHEAD = 3
    for j in range(LOOKAHEAD):
        emit_qk(j)
    for i in range(len(sched)):
        if i + LOOKAHEAD < len(sched):
            emit_qk(i + LOOKAHEAD)
        advance_norm()
    # force-seal the final partial window and flush remaining consumers
    if state['gpos'] > _win_start(state['win_id']):
        seal_window(state['win_id'], partial_end=state['gpos'])
    while pending or consumer_q:
        assert not consumer_q, "consumers left unflushed"
        advance_norm(drain=True)


_CACHED_NC = None


def build_program():
    global _CACHED_NC
    if _CACHED_NC is not None:
        return _CACHED_NC
    nc = bacc.Bacc("TRN2", target_bir_lowering=False, debug=False,
                   num_devices=NCORES)
    qt = nc.dram_tensor("qt", [HEADS_PER_CORE, D, S], BF16,
                        kind="ExternalInput").ap()
    kt = nc.dram_tensor("kt", [PAIRS_PER_CORE, D, S], BF16,
                        kind="ExternalInput").ap()
    v = nc.dram_tensor("v", [PAIRS_PER_CORE, 128, NKT * D], BF16,
                       kind="ExternalInput").ap()
    recd = nc.dram_tensor("recd", [HEADS_PER_CORE, NCH, CH], BF16,
                          kind="Internal").ap()
    ot = nc.dram_tensor("ot", [HEADS_PER_CORE, D, S], BF16,
                        kind="ExternalOutput").ap()
    with tile.TileContext(nc) as tc:
        emit_core_program(tc, qt, kt, v, recd, ot)
    nc.compile()
    _CACHED_NC = nc
    return nc


def shard_inputs(query, key, value):
    """Full inputs -> list of 8 per-core in_maps (host relayout + bf16)."""
    query = np.asarray(query, dtype=np.float32)
    key = np.asarray(key, dtype=np.float32)
    value = np.asarray(value, dtype=np.float32)

    # Q: [S,B,HQ,D] -> [B*HKV, G, D, S]
    qtall = np.ascontiguousarray(
        query.reshape(S, B, HKV, G, D).transpose(1, 2, 3, 4, 0)
    ).reshape(NPAIRS, G, D, S).astype(NP_BF16)
    # K: [S,B,HKV,D] -> [B*HKV, D, S]
    ktall = np.ascontiguousarray(
        key.transpose(1, 2, 3, 0)).reshape(NPAIRS, D, S).astype(NP_BF16)
    # V: [S,B,HKV,D] -> [B*HKV, k_local=128, NKT*D]
    vall = np.ascontiguousarray(
        value.reshape(NKT, 128, B, HKV, D).transpose(2, 3, 1, 0, 4)
    ).reshape(NPAIRS, 128, NKT * D).astype(NP_BF16)

    in_maps = []
    for c in range(NCORES):
        p0 = PAIRS_PER_CORE * c
        p1 = p0 + PAIRS_PER_CORE
        in_maps.append({
            "qt": np.ascontiguousarray(qtall[p0:p1].reshape(HEADS_PER_CORE, D, S)),
            "kt": np.ascontiguousarray(ktall[p0:p1]),
            "v": np.ascontiguousarray(vall[p0:p1]),
        })
    return in_maps


def unshard_output(results):
    """8 per-core {'ot': [8, D, S]} -> full [S, B, HQ, D]."""
    ot = np.stack([np.asarray(r["ot"], dtype=np.float32) for r in results])
    ot = ot.reshape(B, HKV, G, D, S)                   # pairs major -> b, hkv
    out = np.ascontiguousarray(ot.transpose(4, 0, 1, 2, 3))  # [S,B,HKV,G,D]
    return out.reshape(S, B, HQ, D)


def kernel(query, key, value, _trace=False, _return_bkr=False):
    nc = build_program()
    in_maps = shard_inputs(query, key, value)
    bkr = bass_utils.run_bass_kernel_spmd(
        nc, in_maps, core_ids=list(range(NCORES)), trace=_trace)
    out = unshard_output(bkr.results)
    if _return_bkr:
        return out, bkr
    return out


if __name__ == "__main__":
    q = np.random.randn(S, B, HQ, D).astype(np.float32)
    k = np.random.randn(S, B, HKV, D).astype(np.float32)
    vv = np.random.randn(S, B, HKV, D).astype(np.float32)
    o = kernel(q, k, vv)
    print("out", o.shape, o.dtype, float(np.abs(o).max()))


# revision 4
# speedup vs baseline: 1.3647x; 1.3647x over previous
"""Causal GQA attention (S=2048, B=2, HQ=32, HKV=8, D=128) on 8 trn2 cores.

Sharding: the 16 (batch, kv-head) pairs are split 2 per core (data+head
parallel); each pair carries group=4 query heads -> 8 attention heads/core.

v2 dataflow (vs the per-k-tile-ACTIVATE baseline): QK^T staging tiles are
packed back-to-back into a 5-bank PSUM ring (2560 fp32 cols).  The ring is
consumed by WIDE exp ACTIVATEs over bank-aligned windows that alternate
1536/1024 cols, cutting ScalarE per-instruction overhead ~2.6x.  Each
(head, q-chunk, k-tile) region lands at an arbitrary 128-aligned ring
offset; its QK matmuls split at PSUM bank boundaries and its consumers
(AV matmul, DVE row-sum accumulation) slice the window's SBUF exp tile.

The causal mask moved from the DVE to the PE: the diagonal 128x128 block
gets a second accumulating matmul diag(-2000) @ strict_lower_tri, driving
masked logits to -2000 so exp(SCALE*x) underflows to exactly 0.

Softmax denominators are pure-DVE: bf16 element-wise accumulation of each
window slice into a per-chunk sacc tile, folded at chunk end by two
ones-column matmuls into a dedicated 1-bank PSUM row pair (parity-
alternated), then srow -> DMA-reshape [128,8] -> reciprocal -> DRAM ->
partition-broadcast -> multiply -> store, advanced one stage per region
(pipelined across chunks).

PSUM budget: ring 5 banks + sum rows 1 bank + out^T accumulator 2 banks
= 8 exactly.  Host side only re-lays-out data (and casts to bf16).
"""

import numpy as np
import ml_dtypes

import concourse.bass as bass
import concourse.mybir as mybir
import concourse.tile as tile
from concourse import bacc, bass_utils
from concourse.masks import make_lower_triangular

S, B, HQ, HKV, D = 2048, 2, 32, 8, 128
G = HQ // HKV                      # 4 query heads per kv head
NCORES = 8
NPAIRS = B * HKV                   # 16 (batch, kv-head) pairs
PAIRS_PER_CORE = NPAIRS // NCORES  # 2
HEADS_PER_CORE = PAIRS_PER_CORE * G  # 8
SCALE = 1.0 / float(np.sqrt(D))
CH = 1024                          # q-chunk width
NCH = S // CH                      # 2
KT = 128                           # k-tile width
NKT = S // KT                      # 16
RING = 2560                        # 5 PSUM banks of staged logits
NEGC = -2000.0                     # causal mask additive constant

F32 = mybir.dt.float32
BF16 = mybir.dt.bfloat16
NP_BF16 = ml_dtypes.bfloat16


def _win_of(g):
    """Global fill coord -> window id (windows alternate 1536/1024 wide)."""
    cyc, r = divmod(g, RING)
    return 2 * cyc + (0 if r < 1536 else 1)


def _win_start(w):
    return RING * (w // 2) + (0 if w % 2 == 0 else 1536)


def _win_base(w):
    return 0 if w % 2 == 0 else 1536


def _win_width(w):
    return 1536 if w % 2 == 0 else 1024


def emit_core_program(tc, qt, kt, v, recd, ot):
    from contextlib import ExitStack

    nc = tc.nc
    with ExitStack() as ctx:
        _emit_core_program(ctx, tc, nc, qt, kt, v, recd, ot)


def _emit_core_program(ctx, tc, nc, qt, kt, v, recd, ot):
    singles = ctx.enter_context(tc.tile_pool(name="singles", bufs=1))
    kv_pool = ctx.enter_context(tc.tile_pool(name="kv", bufs=2))
    q_pool = ctx.enter_context(tc.tile_pool(name="q", bufs=2))
    pt_pool = ctx.enter_context(tc.tile_pool(name="pt", bufs=4))
    sacc_pool = ctx.enter_context(tc.tile_pool(name="sacc", bufs=2))
    osb_pool = ctx.enter_context(tc.tile_pool(name="osb", bufs=3))
    osb2_pool = ctx.enter_context(tc.tile_pool(name="osb2", bufs=3))
    bcs_pool = ctx.enter_context(tc.tile_pool(name="bcs", bufs=3))
    srow_pool = ctx.enter_context(tc.tile_pool(name="srow", bufs=3))
    srec_pool = ctx.enter_context(tc.tile_pool(name="srec", bufs=3))
    ring_pool = ctx.enter_context(tc.tile_pool(name="ring", bufs=1, space="PSUM"))
    oa_pool = ctx.enter_context(tc.tile_pool(name="oa", bufs=1, space="PSUM"))
    ps_sum = ctx.enter_context(tc.tile_pool(name="ps_sum", bufs=1, space="PSUM"))

    # Constants.
    trislf = singles.tile([128, 128], F32)
    make_lower_triangular(nc, trislf[:], val=1.0, diag=False)  # 1 where p > j
    trisl = singles.tile([128, 128], BF16)
    nc.scalar.copy(out=trisl[:], in_=trislf[:])
    negcf = singles.tile([128, 128], F32)
    nc.gpsimd.memset(negcf[:], NEGC)
    nc.gpsimd.affine_select(            # keep NEGC on the diagonal, 0 off it
        out=negcf[:], in_=negcf[:],
        compare_op=mybir.AluOpType.is_equal, fill=0.0,
        base=0, pattern=[[-1, 128]], channel_multiplier=1,
    )
    negc = singles.tile([128, 128], BF16)
    nc.scalar.copy(out=negc[:], in_=negcf[:])
    onesc = singles.tile([128, 1], BF16)   # ones column (sum-over-k lhsT)
    nc.vector.memset(onesc[:], 1.0)

    ring = ring_pool.tile([128, RING], F32)
    sum_ps = ps_sum.tile([128, 512], F32)
    # srow pulls [33, 512] windows from this bank; the 31 partitions between
    # the two fold rows are never written -- zero once so reads are defined
    nc.vector.memset(sum_ps[:], 0.0)

    kv_tiles = {}

    def ensure_pair(pair):
        if pair in kv_tiles or pair >= PAIRS_PER_CORE:
            return
        kt_sb = kv_pool.tile([D, S], BF16, tag="kt", name=f"kt_{pair}")
        nc.sync.dma_start(out=kt_sb[:], in_=kt[pair])
        v_sb = kv_pool.tile([128, NKT * D], BF16, tag="v", name=f"v_{pair}")
        nc.sync.dma_start(out=v_sb[:], in_=v[pair])
        kv_tiles[pair] = (kt_sb, v_sb)

    q_tiles = {}

    def ensure_head(head):
        if head in q_tiles or head >= HEADS_PER_CORE:
            return
        q_sb = q_pool.tile([D, S], BF16, tag="q", name=f"q_{head}")
        nc.sync.dma_start(out=q_sb[:], in_=qt[head])
        q_tiles[head] = q_sb

    # Flat schedule of regions: (head, chunk, kti)
    sched = []
    for head in range(HEADS_PER_CORE):
        for c in range(NCH):
            for kti in range(8 * c + 8):
                sched.append((head, c, kti))

    # --- mutable emission state ---
    state = dict(gpos=0, win_id=0, sealed=-1)
    win_tiles = {}     # win id -> (pt tile, global window start)
    region_meta = []   # per region: (ga, qa, width)
    consumer_q = []    # (last_win_needed, region index)
    oacc = {}          # (head, c) -> psum accumulator tile
    av_started = {}    # (head, c) -> set of banks already started
    saccs = {}         # (head, c) -> sacc tile
    pending = []       # chunk-tail normalization pipeline entries
    tail_count = [0]

    def seal_window(w, partial_end=None):
        gs = _win_start(w)
        width = (partial_end - gs) if partial_end is not None else _win_width(w)
        base = _win_base(w)
        ptw = pt_pool.tile([128, 1536], BF16, tag="pt", name=f"pt_{w}")
        nc.scalar.activation(
            ptw[:, 0:width], ring[:, base:base + width],
            mybir.ActivationFunctionType.Exp, scale=SCALE)
        win_tiles[w] = (ptw, gs)
        state['sealed'] = w
        # Flush consumers lagging ONE window behind the seal: emitting a
        # window's AV matmuls at its own seal would park them at the head of
        # the PE FIFO waiting out the full exp ACTIVATE (~1.5us, every
        # window).  The one-window delay gives the ACTIVATE a whole window
        # of QK matmuls as head start.
        while consumer_q and consumer_q[0][0] <= state['sealed'] - 1:
            _, ridx = consumer_q.pop(0)
            emit_consumers(ridx)

    def flush_all_consumers():
        while consumer_q and consumer_q[0][0] <= state['sealed']:
            _, ridx = consumer_q.pop(0)
            emit_consumers(ridx)

    def maybe_seal():
        while state['gpos'] >= _win_start(state['win_id']) + _win_width(state['win_id']):
            seal_window(state['win_id'])
            state['win_id'] += 1

    def emit_qk(i):
        head, c, kti = sched[i]
        if c == 0 and kti == 0:
            pair = head // G
            ensure_pair(pair + 1)
            ensure_head(head + 1)
        kt_sb, _ = kv_tiles[head // G]
        q_sb = q_tiles[head]
        off = max(0, 128 * kti - CH * c)
        w = CH - off
        has_diag = 128 * kti >= CH * c
        ga = state['gpos']
        region_meta.append((ga, off, w))
        # spans: split [ga, ga+w) at RING wrap
        spans = []
        g0 = ga
        rem = w
        while rem > 0:
            ra = g0 % RING
            ln = min(rem, RING - ra)
            spans.append((g0, ra, ln))
            g0 += ln
            rem -= ln
        first = True
        for (gsp, ra, ln) in spans:
            # pieces: split at PSUM bank boundaries (512 multiples of ring coord)
            p = 0
            while p < ln:
                seg = min(ln - p, 512 - (ra + p) % 512)
                a = ra + p
                qa = off + (gsp - ga) + p
                diag_here = has_diag and first and p == 0
                nc.tensor.matmul(
                    out=ring[:, a:a + seg],
                    lhsT=kt_sb[:, 128 * kti:128 * (kti + 1)],
                    rhs=q_sb[:, CH * c + qa:CH * c + qa + seg],
                    start=True, stop=(not diag_here),
                )
                if diag_here:
                    nc.tensor.matmul(
                        out=ring[:, a:a + 128],
                        lhsT=negc[:], rhs=trisl[:],
                        start=False, stop=True,
                    )
                p += seg
            first = False
        state['gpos'] += w
        # queue consumers keyed on the last window this region touches
        last_win = _win_of(state['gpos'] - 1)
        consumer_q.append((last_win, i))
        maybe_seal()

    def _pt_slice(g):
        """global ring coord -> (pt tile, rel offset, cols remaining in window)"""
        wd = _win_of(g)
        ptw, gs = win_tiles[wd]
        rel = g - gs
        return ptw, rel, _win_width(wd) - rel

    def emit_consumers(i):
        head, c, kti = sched[i]
        _, v_sb = kv_tiles[head // G]
        ga, qa, w = region_meta[i]
        qb = qa + w
        last = kti == 8 * c + 7
        key = (head, c)

        # --- denominators: sacc += exp slice (bf16, DVE) ---
        if kti == 0:
            sacc = sacc_pool.tile([128, CH], BF16, tag="sacc",
                                  name=f"sacc_{head}_{c}")
            saccs[key] = sacc
            first_sacc = True
        else:
            sacc = saccs[key]
            first_sacc = False
        q0 = qa
        while q0 < qb:
            ptw, rel, win_rem = _pt_slice(ga + (q0 - qa))
            q1 = min(qb, q0 + win_rem)
            if first_sacc:
                nc.vector.tensor_copy(sacc[:, q0:q1], ptw[:, rel:rel + (q1 - q0)])
            else:
                nc.vector.tensor_add(
                    sacc[:, q0:q1], sacc[:, q0:q1], ptw[:, rel:rel + (q1 - q0)])
            q0 = q1

        # --- out^T += V^T P^T ---
        if kti == 0:
            oacc[key] = oa_pool.tile([128, CH], F32, tag="oacc",
                                     name=f"oa_{head}_{c}")
            av_started[key] = set()
        oa = oacc[key]
        started = av_started[key]
        q0 = qa
        while q0 < qb:
            ptw, rel, win_rem = _pt_slice(ga + (q0 - qa))
            # split at q-bank boundary (512) and window end
            q1 = min(qb, q0 + win_rem, (q0 // 512 + 1) * 512)
            bank = q0 // 512
            stop_kti = 8 * c + 3 if bank == 0 else 8 * c + 7
            nc.tensor.matmul(
                out=oa[:, q0:q1],
                lhsT=v_sb[:, D * kti:D * (kti + 1)],
                rhs=ptw[:, rel:rel + (q1 - q0)],
                start=(bank not in started),
                stop=(kti == stop_kti and (q1 == qb or q1 % 512 == 0)),
            )
            started.add(bank)
            q0 = q1

        if last:
            # chunk tail: fold sacc into PSUM sum rows; evacuate oacc
            sacc = saccs.pop(key)
            rb = 64 * (tail_count[0] % 2)
            tail_count[0] += 1
            nc.tensor.matmul(
                out=sum_ps[rb:rb + 1, 0:512], lhsT=onesc[:],
                rhs=sacc[:, 0:512], start=True, stop=True,
                tile_position=(0, rb),
            )
            nc.tensor.matmul(
                out=sum_ps[rb + 32:rb + 33, 0:512], lhsT=onesc[:],
                rhs=sacc[:, 512:CH], start=True, stop=True,
                tile_position=(0, rb + 32),
            )
            oa = oacc.pop(key)
            av_started.pop(key)
            osb = osb_pool.tile([128, CH], BF16, tag="osb",
                                name=f"osb_{head}_{c}")
            nc.vector.tensor_copy(osb[:], oa[:])
            pending.append(dict(head=head, c=c, stage=0, osb=osb,
                                rb=rb, born=i))

    def advance_norm(drain=False):
        for ent in list(pending):
            head, c, st = ent["head"], ent["c"], ent["stage"]
            if st == 0:
                rb = ent["rb"]
                srow = srow_pool.tile([33, 512], F32, tag="srow",
                                      name=f"srow_{head}_{c}")
                nc.vector.tensor_copy(srow[:], sum_ps[rb:rb + 33, 0:512])
                srec = srec_pool.tile([128, NCH * 4], F32, tag="srec",
                                      name=f"srec_{head}_{c}")
                nc.sync.dma_start(out=srec[0:64, :], in_=srow[0:1, :])
                nc.sync.dma_start(out=srec[64:128, :], in_=srow[32:33, :])
                ent["srec"] = srec
            elif st == 1:
                srec2 = srec_pool.tile([128, NCH * 4], BF16, tag="srec2",
                                       name=f"srec2_{head}_{c}")
                with nc.allow_low_precision(reason="1/sum broadcast in bf16"):
                    nc.vector.reciprocal(out=srec2[:], in_=ent["srec"][:])
                nc.sync.dma_start(out=recd[head, c], in_=srec2[:])
            elif st == 2:
                bcs = bcs_pool.tile([128, CH], BF16, tag="bcs",
                                    name=f"bcs_{head}_{c}")
                nc.sync.dma_start(
                    out=bcs[:], in_=recd[head, c].partition_broadcast(128))
                ent["bcs"] = bcs
            elif st == 3:
                osb2 = osb2_pool.tile([128, CH], BF16, tag="osb2",
                                      name=f"osb2_{head}_{c}")
                nc.vector.tensor_mul(osb2[:], ent["osb"][:], ent["bcs"][:])
                nc.sync.dma_start(
                    out=ot[head][:, CH * c:CH * (c + 1)], in_=osb2[:])
                pending.remove(ent)
            ent["stage"] = st + 1
            break

    # --- main emission loop: QK runs 3 regions ahead of consumers ---
    ensure_pair(0)
    ensure_head(0)
    LOOKAHEAD = 3
    for j in range(LOOKAHEAD):
        emit_qk(j)
    for i in range(len(sched)):
        if i + LOOKAHEAD < len(sched):
            emit_qk(i + LOOKAHEAD)
        advance_norm()
    # force-seal the final partial window and flush remaining consumers
    if state['gpos'] > _win_start(state['win_id']):
        seal_window(state['win_id'], partial_end=state['gpos'])
    flush_all_consumers()
    assert not consumer_q, "consumers left unflushed"
    while pending:
        advance_norm(drain=True)


_CACHED_NC = None


def build_program():
    global _CACHED_NC
    if _CACHED_NC is not None:
        return _CACHED_NC
    nc = bacc.Bacc("TRN2", target_bir_lowering=False, debug=False,
                   num_devices=NCORES)
    qt = nc.dram_tensor("qt", [HEADS_PER_CORE, D, S], BF16,
                        kind="ExternalInput").ap()
    kt = nc.dram_tensor("kt", [PAIRS_PER_CORE, D, S], BF16,
                        kind="ExternalInput").ap()
    v = nc.dram_tensor("v", [PAIRS_PER_CORE, 128, NKT * D], BF16,
                       kind="ExternalInput").ap()
    recd = nc.dram_tensor("recd", [HEADS_PER_CORE, NCH, CH], BF16,
                          kind="Internal").ap()
    ot = nc.dram_tensor("ot", [HEADS_PER_CORE, D, S], BF16,
                        kind="ExternalOutput").ap()
    with tile.TileContext(nc) as tc:
        emit_core_program(tc, qt, kt, v, recd, ot)
    nc.compile()
    _CACHED_NC = nc
    return nc


def shard_inputs(query, key, value):
    """Full inputs -> list of 8 per-core in_maps (host relayout + bf16)."""
    query = np.asarray(query, dtype=np.float32)
    key = np.asarray(key, dtype=np.float32)
    value = np.asarray(value, dtype=np.float32)

    # Q: [S,B,HQ,D] -> [B*HKV, G, D, S]
    qtall = np.ascontiguousarray(
        query.reshape(S, B, HKV, G, D).transpose(1, 2, 3, 4, 0)
    ).reshape(NPAIRS, G, D, S).astype(NP_BF16)
    # K: [S,B,HKV,D] -> [B*HKV, D, S]
    ktall = np.ascontiguousarray(
        key.transpose(1, 2, 3, 0)).reshape(NPAIRS, D, S).astype(NP_BF16)
    # V: [S,B,HKV,D] -> [B*HKV, k_local=128, NKT*D]
    vall = np.ascontiguousarray(
        value.reshape(NKT, 128, B, HKV, D).transpose(2, 3, 1, 0, 4)
    ).reshape(NPAIRS, 128, NKT * D).astype(NP_BF16)

    in_maps = []
    for c in range(NCORES):
        p0 = PAIRS_PER_CORE * c
        p1 = p0 + PAIRS_PER_CORE
        in_maps.append({
            "qt": np.ascontiguousarray(qtall[p0:p1].reshape(HEADS_PER_CORE, D, S)),
            "kt": np.ascontiguousarray(ktall[p0:p1]),
            "v": np.ascontiguousarray(vall[p0:p1]),
        })
    return in_maps


def unshard_output(results):
    """8 per-core {'ot': [8, D, S]} -> full [S, B, HQ, D]."""
    ot = np.stack([np.asarray(r["ot"], dtype=np.float32) for r in results])
    ot = ot.reshape(B, HKV, G, D, S)                   # pairs major -> b, hkv
    out = np.ascontiguousarray(ot.transpose(4, 0, 1, 2, 3))  # [S,B,HKV,G,D]
    return out.reshape(S, B, HQ, D)


def kernel(query, key, value, _trace=False, _return_bkr=False):
    nc = build_program()
    in_maps = shard_inputs(query, key, value)
    bkr = bass_utils.run_bass_kernel_spmd(
        nc, in_maps, core_ids=list(range(NCORES)), trace=_trace)
    out = unshard_output(bkr.results)
    if _return_bkr:
        return out, bkr
    return out


if __name__ == "__main__":
    q = np.random.randn(S, B, HQ, D).astype(np.float32)
    k = np.random.randn(S, B, HKV, D).astype(np.float32)
    vv = np.random.randn(S, B, HKV, D).astype(np.float32)
    o = kernel(q, k, vv)
    print("out", o.shape, o.dtype, float(np.abs(o).max()))


# revision 6
# speedup vs baseline: 1.3659x; 1.0009x over previous
"""Causal GQA attention (S=2048, B=2, HQ=32, HKV=8, D=128) on 8 trn2 cores.

Sharding: the 16 (batch, kv-head) pairs are split 2 per core (data+head
parallel); each pair carries group=4 query heads -> 8 attention heads/core.

v2 dataflow (vs the per-k-tile-ACTIVATE baseline): QK^T staging tiles are
packed back-to-back into a 5-bank PSUM ring (2560 fp32 cols).  The ring is
consumed by WIDE exp ACTIVATEs over bank-aligned windows that alternate
1536/1024 cols, cutting ScalarE per-instruction overhead ~2.6x.  Each
(head, q-chunk, k-tile) region lands at an arbitrary 128-aligned ring
offset; its QK matmuls split at PSUM bank boundaries and its consumers
(AV matmul, DVE row-sum accumulation) slice the window's SBUF exp tile.

The causal mask moved from the DVE to the PE: the diagonal 128x128 block
gets a second accumulating matmul diag(-2000) @ strict_lower_tri, driving
masked logits to -2000 so exp(SCALE*x) underflows to exactly 0.

Softmax denominators are pure-DVE: bf16 element-wise accumulation of each
window slice into a per-chunk sacc tile, folded at chunk end by two
ones-column matmuls into a dedicated 1-bank PSUM row pair (parity-
alternated), then srow -> DMA-reshape [128,8] -> reciprocal -> DRAM ->
partition-broadcast -> multiply -> store, advanced one stage per region
(pipelined across chunks).

PSUM budget: ring 5 banks + sum rows 1 bank + out^T accumulator 2 banks
= 8 exactly.  Host side only re-lays-out data (and casts to bf16).
"""

import numpy as np
import ml_dtypes

import concourse.bass as bass
import concourse.mybir as mybir
import concourse.tile as tile
from concourse import bacc, bass_utils
from concourse.masks import make_lower_triangular

S, B, HQ, HKV, D = 2048, 2, 32, 8, 128
G = HQ // HKV                      # 4 query heads per kv head
NCORES = 8
NPAIRS = B * HKV                   # 16 (batch, kv-head) pairs
PAIRS_PER_CORE = NPAIRS // NCORES  # 2
HEADS_PER_CORE = PAIRS_PER_CORE * G  # 8
SCALE = 1.0 / float(np.sqrt(D))
CH = 1024                          # q-chunk width
NCH = S // CH                      # 2
KT = 128                           # k-tile width
NKT = S // KT                      # 16
RING = 2560                        # 5 PSUM banks of staged logits
NEGC = -2000.0                     # causal mask additive constant

F32 = mybir.dt.float32
BF16 = mybir.dt.bfloat16
NP_BF16 = ml_dtypes.bfloat16


def _win_of(g):
    """Global fill coord -> window id (windows alternate 1536/1024 wide)."""
    cyc, r = divmod(g, RING)
    return 2 * cyc + (0 if r < 1536 else 1)


def _win_start(w):
    return RING * (w // 2) + (0 if w % 2 == 0 else 1536)


def _win_base(w):
    return 0 if w % 2 == 0 else 1536


def _win_width(w):
    return 1536 if w % 2 == 0 else 1024


def emit_core_program(tc, qt, kt, v, recd, ot):
    from contextlib import ExitStack

    nc = tc.nc
    with ExitStack() as ctx:
        _emit_core_program(ctx, tc, nc, qt, kt, v, recd, ot)


def _emit_core_program(ctx, tc, nc, qt, kt, v, recd, ot):
    singles = ctx.enter_context(tc.tile_pool(name="singles", bufs=1))
    kv_pool = ctx.enter_context(tc.tile_pool(name="kv", bufs=2))
    q_pool = ctx.enter_context(tc.tile_pool(name="q", bufs=2))
    pt_pool = ctx.enter_context(tc.tile_pool(name="pt", bufs=5))
    sacc_pool = ctx.enter_context(tc.tile_pool(name="sacc", bufs=2))
    osb_pool = ctx.enter_context(tc.tile_pool(name="osb", bufs=3))
    osb2_pool = ctx.enter_context(tc.tile_pool(name="osb2", bufs=3))
    bcs_pool = ctx.enter_context(tc.tile_pool(name="bcs", bufs=3))
    srow_pool = ctx.enter_context(tc.tile_pool(name="srow", bufs=3))
    srec_pool = ctx.enter_context(tc.tile_pool(name="srec", bufs=3))
    ring_pool = ctx.enter_context(tc.tile_pool(name="ring", bufs=1, space="PSUM"))
    oa_pool = ctx.enter_context(tc.tile_pool(name="oa", bufs=1, space="PSUM"))
    ps_sum = ctx.enter_context(tc.tile_pool(name="ps_sum", bufs=1, space="PSUM"))

    # Constants.
    trislf = singles.tile([128, 128], F32)
    make_lower_triangular(nc, trislf[:], val=1.0, diag=False)  # 1 where p > j
    trisl = singles.tile([128, 128], BF16)
    nc.scalar.copy(out=trisl[:], in_=trislf[:])
    negcf = singles.tile([128, 128], F32)
    nc.gpsimd.memset(negcf[:], NEGC)
    nc.gpsimd.affine_select(            # keep NEGC on the diagonal, 0 off it
        out=negcf[:], in_=negcf[:],
        compare_op=mybir.AluOpType.is_equal, fill=0.0,
        base=0, pattern=[[-1, 128]], channel_multiplier=1,
    )
    negc = singles.tile([128, 128], BF16)
    nc.scalar.copy(out=negc[:], in_=negcf[:])
    onesc = singles.tile([128, 1], BF16)   # ones column (sum-over-k lhsT)
    nc.vector.memset(onesc[:], 1.0)

    ring = ring_pool.tile([128, RING], F32)
    sum_ps = ps_sum.tile([128, 512], F32)
    # srow pulls [33, 512] windows from this bank; the 31 partitions between
    # the two fold rows are never written -- zero once so reads are defined
    nc.vector.memset(sum_ps[:], 0.0)

    kv_tiles = {}

    def ensure_pair(pair):
        if pair in kv_tiles or pair >= PAIRS_PER_CORE:
            return
        kt_sb = kv_pool.tile([D, S], BF16, tag="kt", name=f"kt_{pair}")
        nc.sync.dma_start(out=kt_sb[:], in_=kt[pair])
        v_sb = kv_pool.tile([128, NKT * D], BF16, tag="v", name=f"v_{pair}")
        nc.sync.dma_start(out=v_sb[:], in_=v[pair])
        kv_tiles[pair] = (kt_sb, v_sb)

    q_tiles = {}

    def ensure_head(head):
        if head in q_tiles or head >= HEADS_PER_CORE:
            return
        q_sb = q_pool.tile([D, S], BF16, tag="q", name=f"q_{head}")
        nc.sync.dma_start(out=q_sb[:], in_=qt[head])
        q_tiles[head] = q_sb

    # Flat schedule of regions: (head, chunk, kti)
    sched = []
    for head in range(HEADS_PER_CORE):
        for c in range(NCH):
            for kti in range(8 * c + 8):
                sched.append((head, c, kti))

    # --- mutable emission state ---
    state = dict(gpos=0, win_id=0, sealed=-1)
    win_tiles = {}     # win id -> (pt tile, global window start)
    region_meta = []   # per region: (ga, qa, width)
    consumer_q = []    # (last_win_needed, region index)
    oacc = {}          # (head, c) -> psum accumulator tile
    av_started = {}    # (head, c) -> set of banks already started
    saccs = {}         # (head, c) -> sacc tile
    pending = []       # chunk-tail normalization pipeline entries
    tail_count = [0]

    def seal_window(w, partial_end=None):
        gs = _win_start(w)
        width = (partial_end - gs) if partial_end is not None else _win_width(w)
        base = _win_base(w)
        ptw = pt_pool.tile([128, 1536], BF16, tag="pt", name=f"pt_{w}")
        nc.scalar.activation(
            ptw[:, 0:width], ring[:, base:base + width],
            mybir.ActivationFunctionType.Exp, scale=SCALE)
        win_tiles[w] = (ptw, gs)
        state['sealed'] = w
        # Flush consumers lagging ONE window behind the seal: emitting a
        # window's AV matmuls at its own seal would park them at the head of
        # the PE FIFO waiting out the full exp ACTIVATE (~1.5us, every
        # window).  The one-window delay gives the ACTIVATE a whole window
        # of QK matmuls as head start.
        while consumer_q and consumer_q[0][0] <= state['sealed'] - 2:
            _, ridx = consumer_q.pop(0)
            emit_consumers(ridx)

    def flush_all_consumers():
        while consumer_q and consumer_q[0][0] <= state['sealed']:
            _, ridx = consumer_q.pop(0)
            emit_consumers(ridx)

    def maybe_seal():
        while state['gpos'] >= _win_start(state['win_id']) + _win_width(state['win_id']):
            seal_window(state['win_id'])
            state['win_id'] += 1

    def emit_qk(i):
        head, c, kti = sched[i]
        if c == 0 and kti == 0:
            pair = head // G
            ensure_pair(pair + 1)
            ensure_head(head + 1)
        kt_sb, _ = kv_tiles[head // G]
        q_sb = q_tiles[head]
        off = max(0, 128 * kti - CH * c)
        w = CH - off
        has_diag = 128 * kti >= CH * c
        ga = state['gpos']
        region_meta.append((ga, off, w))
        # spans: split [ga, ga+w) at RING wrap
        spans = []
        g0 = ga
        rem = w
        while rem > 0:
            ra = g0 % RING
            ln = min(rem, RING - ra)
            spans.append((g0, ra, ln))
            g0 += ln
            rem -= ln
        first = True
        for (gsp, ra, ln) in spans:
            # pieces: split at PSUM bank boundaries (512 multiples of ring coord)
            p = 0
            while p < ln:
                seg = min(ln - p, 512 - (ra + p) % 512)
                a = ra + p
                qa = off + (gsp - ga) + p
                diag_here = has_diag and first and p == 0
                nc.tensor.matmul(
                    out=ring[:, a:a + seg],
                    lhsT=kt_sb[:, 128 * kti:128 * (kti + 1)],
                    rhs=q_sb[:, CH * c + qa:CH * c + qa + seg],
                    start=True, stop=(not diag_here),
                )
                if diag_here:
                    nc.tensor.matmul(
                        out=ring[:, a:a + 128],
                        lhsT=negc[:], rhs=trisl[:],
                        start=False, stop=True,
                    )
                p += seg
            first = False
        state['gpos'] += w
        # queue consumers keyed on the last window this region touches
        last_win = _win_of(state['gpos'] - 1)
        consumer_q.append((last_win, i))
        maybe_seal()

    def _pt_slice(g):
        """global ring coord -> (pt tile, rel offset, cols remaining in window)"""
        wd = _win_of(g)
        ptw, gs = win_tiles[wd]
        rel = g - gs
        return ptw, rel, _win_width(wd) - rel

    def emit_consumers(i):
        head, c, kti = sched[i]
        _, v_sb = kv_tiles[head // G]
        ga, qa, w = region_meta[i]
        qb = qa + w
        last = kti == 8 * c + 7
        key = (head, c)

        # --- denominators: sacc += exp slice (bf16, DVE) ---
        if kti == 0:
            sacc = sacc_pool.tile([128, CH], BF16, tag="sacc",
                                  name=f"sacc_{head}_{c}")
            saccs[key] = sacc
            first_sacc = True
        else:
            sacc = saccs[key]
            first_sacc = False
        q0 = qa
        while q0 < qb:
            ptw, rel, win_rem = _pt_slice(ga + (q0 - qa))
            q1 = min(qb, q0 + win_rem)
            if first_sacc:
                nc.vector.tensor_copy(sacc[:, q0:q1], ptw[:, rel:rel + (q1 - q0)])
            else:
                nc.vector.tensor_add(
                    sacc[:, q0:q1], sacc[:, q0:q1], ptw[:, rel:rel + (q1 - q0)])
            q0 = q1

        # --- out^T += V^T P^T ---
        if kti == 0:
            oacc[key] = oa_pool.tile([128, CH], F32, tag="oacc",
                                     name=f"oa_{head}_{c}")
            av_started[key] = set()
        oa = oacc[key]
        started = av_started[key]
        q0 = qa
        while q0 < qb:
            ptw, rel, win_rem = _pt_slice(ga + (q0 - qa))
            # split at q-bank boundary (512) and window end
            q1 = min(qb, q0 + win_rem, (q0 // 512 + 1) * 512)
            bank = q0 // 512
            stop_kti = 8 * c + 3 if bank == 0 else 8 * c + 7
            nc.tensor.matmul(
                out=oa[:, q0:q1],
                lhsT=v_sb[:, D * kti:D * (kti + 1)],
                rhs=ptw[:, rel:rel + (q1 - q0)],
                start=(bank not in started),
                stop=(kti == stop_kti and (q1 == qb or q1 % 512 == 0)),
            )
            started.add(bank)
            q0 = q1

        if last:
            # chunk tail: fold sacc into PSUM sum rows; evacuate oacc
            sacc = saccs.pop(key)
            rb = 64 * (tail_count[0] % 2)
            tail_count[0] += 1
            nc.tensor.matmul(
                out=sum_ps[rb:rb + 1, 0:512], lhsT=onesc[:],
                rhs=sacc[:, 0:512], start=True, stop=True,
                tile_position=(0, rb),
            )
            nc.tensor.matmul(
                out=sum_ps[rb + 32:rb + 33, 0:512], lhsT=onesc[:],
                rhs=sacc[:, 512:CH], start=True, stop=True,
                tile_position=(0, rb + 32),
            )
            oa = oacc.pop(key)
            av_started.pop(key)
            osb = osb_pool.tile([128, CH], BF16, tag="osb",
                                name=f"osb_{head}_{c}")
            nc.vector.tensor_copy(osb[:], oa[:])
            pending.append(dict(head=head, c=c, stage=0, osb=osb,
                                rb=rb, born=i))

    def advance_norm(drain=False):
        for ent in list(pending):
            head, c, st = ent["head"], ent["c"], ent["stage"]
            if st == 0:
                rb = ent["rb"]
                srow = srow_pool.tile([33, 512], F32, tag="srow",
                                      name=f"srow_{head}_{c}")
                nc.vector.tensor_copy(srow[:], sum_ps[rb:rb + 33, 0:512])
                srec = srec_pool.tile([128, NCH * 4], F32, tag="srec",
                                      name=f"srec_{head}_{c}")
                nc.sync.dma_start(out=srec[0:64, :], in_=srow[0:1, :])
                nc.sync.dma_start(out=srec[64:128, :], in_=srow[32:33, :])
                ent["srec"] = srec
            elif st == 1:
                srec2 = srec_pool.tile([128, NCH * 4], BF16, tag="srec2",
                                       name=f"srec2_{head}_{c}")
                with nc.allow_low_precision(reason="1/sum broadcast in bf16"):
                    nc.vector.reciprocal(out=srec2[:], in_=ent["srec"][:])
                nc.sync.dma_start(out=recd[head, c], in_=srec2[:])
            elif st == 2:
                bcs = bcs_pool.tile([128, CH], BF16, tag="bcs",
                                    name=f"bcs_{head}_{c}")
                nc.sync.dma_start(
                    out=bcs[:], in_=recd[head, c].partition_broadcast(128))
                ent["bcs"] = bcs
            elif st == 3:
                osb2 = osb2_pool.tile([128, CH], BF16, tag="osb2",
                                      name=f"osb2_{head}_{c}")
                nc.vector.tensor_mul(osb2[:], ent["osb"][:], ent["bcs"][:])
                nc.sync.dma_start(
                    out=ot[head][:, CH * c:CH * (c + 1)], in_=osb2[:])
                pending.remove(ent)
            ent["stage"] = st + 1
            break

    # --- main emission loop: QK runs 3 regions ahead of consumers ---
    ensure_pair(0)
    ensure_head(0)
    LOOKAHEAD = 3
    for j in range(LOOKAHEAD):
        emit_qk(j)
    for i in range(len(sched)):
        if i + LOOKAHEAD < len(sched):
            emit_qk(i + LOOKAHEAD)
        advance_norm()
    # force-seal the final partial window and flush remaining consumers
    if state['gpos'] > _win_start(state['win_id']):
        seal_window(state['win_id'], partial_end=state['gpos'])
    flush_all_consumers()
    assert not consumer_q, "consumers left unflushed"
    while pending:
        advance_norm(drain=True)


_CACHED_NC = None


def build_program():
    global _CACHED_NC
    if _CACHED_NC is not None:
        return _CACHED_NC
    nc = bacc.Bacc("TRN2", target_bir_lowering=False, debug=False,
                   num_devices=NCORES)
    qt = nc.dram_tensor("qt", [HEADS_PER_CORE, D, S], BF16,
                        kind="ExternalInput").ap()
    kt = nc.dram_tensor("kt", [PAIRS_PER_CORE, D, S], BF16,
                        kind="ExternalInput").ap()
    v = nc.dram_tensor("v", [PAIRS_PER_CORE, 128, NKT * D], BF16,
                       kind="ExternalInput").ap()
    recd = nc.dram_tensor("recd", [HEADS_PER_CORE, NCH, CH], BF16,
                          kind="Internal").ap()
    ot = nc.dram_tensor("ot", [HEADS_PER_CORE, D, S], BF16,
                        kind="ExternalOutput").ap()
    with tile.TileContext(nc) as tc:
        emit_core_program(tc, qt, kt, v, recd, ot)
    nc.compile()
    _CACHED_NC = nc
    return nc


def shard_inputs(query, key, value):
    """Full inputs -> list of 8 per-core in_maps (host relayout + bf16)."""
    query = np.asarray(query, dtype=np.float32)
    key = np.asarray(key, dtype=np.float32)
    value = np.asarray(value, dtype=np.float32)

    # Q: [S,B,HQ,D] -> [B*HKV, G, D, S]
    qtall = np.ascontiguousarray(
        query.reshape(S, B, HKV, G, D).transpose(1, 2, 3, 4, 0)
    ).reshape(NPAIRS, G, D, S).astype(NP_BF16)
    # K: [S,B,HKV,D] -> [B*HKV, D, S]
    ktall = np.ascontiguousarray(
        key.transpose(1, 2, 3, 0)).reshape(NPAIRS, D, S).astype(NP_BF16)
    # V: [S,B,HKV,D] -> [B*HKV, k_local=128, NKT*D]
    vall = np.ascontiguousarray(
        value.reshape(NKT, 128, B, HKV, D).transpose(2, 3, 1, 0, 4)
    ).reshape(NPAIRS, 128, NKT * D).astype(NP_BF16)

    in_maps = []
    for c in range(NCORES):
        p0 = PAIRS_PER_CORE * c
        p1 = p0 + PAIRS_PER_CORE
        in_maps.append({
            "qt": np.ascontiguousarray(qtall[p0:p1].reshape(HEADS_PER_CORE, D, S)),
            "kt": np.ascontiguousarray(ktall[p0:p1]),
            "v": np.ascontiguousarray(vall[p0:p1]),
        })
    return in_maps


def unshard_output(results):
    """8 per-core {'ot': [8, D, S]} -> full [S, B, HQ, D]."""
    ot = np.stack([np.asarray(r["ot"], dtype=np.float32) for r in results])
    ot = ot.reshape(B, HKV, G, D, S)                   # pairs major -> b, hkv
    out = np.ascontiguousarray(ot.transpose(4, 0, 1, 2, 3))  # [S,B,HKV,G,D]
    return out.reshape(S, B, HQ, D)


def kernel(query, key, value, _trace=False, _return_bkr=False):
    nc = build_program()
    in_maps = shard_inputs(query, key, value)
    bkr = bass_utils.run_bass_kernel_spmd(
        nc, in_maps, core_ids=list(range(NCORES)), trace=_trace)
    out = unshard_output(bkr.results)
    if _return_bkr:
        return out, bkr
    return out


if __name__ == "__main__":
    q = np.random.randn(S, B, HQ, D).astype(np.float32)
    k = np.random.randn(S, B, HKV, D).astype(np.float32)
    vv = np.random.randn(S, B, HKV, D).astype(np.float32)
    o = kernel(q, k, vv)
    print("out", o.shape, o.dtype, float(np.abs(o).max()))


# revision 7
# speedup vs baseline: 1.6344x; 1.1966x over previous
"""Causal GQA attention (S=2048, B=2, HQ=32, HKV=8, D=128) on 8 trn2 cores.

Sharding: the 16 (batch, kv-head) pairs are split 2 per core (data+head
parallel). Each pair carries group=4 query heads -> 8 attention heads/core.

Per head the device kernel walks two 1024-wide q-chunks; for each chunk it
streams the causal k-tiles (128 wide): S^T = (Q K^T)^T lands in a 2-bank
PSUM staging tile (k on partitions, q on the free axis), one wide ACTIVATE
exponentiates it into SBUF (P^T, bf16), the 128x128 diagonal block is
masked by a triangular multiply, and V-stationary matmuls accumulate
out^T = V^T P^T into a persistent 2-bank PSUM accumulator. All matmul
operands are bf16 (1 col/cycle on the PE at full clock) and every matmul
is <=512 moving columns so no instruction straddles a PSUM bank.

Softmax denominators: k-tiles with kti%4==0 go straight to the PE as
ones-column matmuls into a shared PSUM sum bank (kti==0 opens the
accumulation with full chunk width); the other k-tiles are element-wise
accumulated on the DVE into a bf16 partial-sum tile, which a single pair
of ones-matmuls folds into the same PSUM rows at chunk end. This keeps
both PE and DVE under the ScalarE exp floor, which is the roofline here
(1 elem/lane/cycle @ 1.2 GHz over ~17.4M causal logits/core).

Chunk tails are software-pipelined: the accumulator is evacuated to SBUF
(bf16) immediately so the next chunk's matmuls can reuse the PSUM bank,
then sums -> SBUF -> DMA-reshape [128,8] -> reciprocal -> DRAM ->
partition-broadcast -> multiply -> store advances one stage per k-tile
iteration, several chunks in flight.

Host side only re-lays-out data (and casts to bf16): Q/K as [d, s], V as
[k_local, ktile*d]; the returned out^T [d, s] is transposed/cast back.
"""

import numpy as np
import ml_dtypes

import concourse.bass as bass
import concourse.mybir as mybir
import concourse.tile as tile
from concourse import bacc, bass_utils
from concourse.masks import make_upper_triangular

S, B, HQ, HKV, D = 2048, 2, 32, 8, 128
G = HQ // HKV                      # 4 query heads per kv head
NCORES = 8
NPAIRS = B * HKV                   # 16 (batch, kv-head) pairs
PAIRS_PER_CORE = NPAIRS // NCORES  # 2
HEADS_PER_CORE = PAIRS_PER_CORE * G  # 8
SCALE = 1.0 / float(np.sqrt(D))
CH = 1024                          # q-chunk width (2 PSUM banks)
NCH = S // CH                      # 2
KT = 128                           # k-tile (partition) width
NKT = S // KT                      # 16

F32 = mybir.dt.float32
BF16 = mybir.dt.bfloat16
NP_BF16 = ml_dtypes.bfloat16


def _segs(off):
    """Split chunk cols [off, CH) into <=512 pieces that don't straddle
    the 512 boundary (one PSUM bank per matmul)."""
    if off < 512:
        return [(off, 512), (512, CH)]
    return [(off, CH)]


def _sum_rc(row_base, s0, s1):
    """Map chunk cols [s0, s1) to (row, col range) in the 512-wide sum
    bank: lo half at row_base, hi half at row_base+32."""
    if s0 < 512:
        return row_base, s0, s1
    return row_base + 32, s0 - 512, s1 - 512


def emit_core_program(tc, qt, kt, v, recd, ot):
    from contextlib import ExitStack

    nc = tc.nc
    with ExitStack() as ctx:
        _emit_core_program(ctx, tc, nc, qt, kt, v, recd, ot)


def _emit_core_program(ctx, tc, nc, qt, kt, v, recd, ot):
    singles = ctx.enter_context(tc.tile_pool(name="singles", bufs=1))
    kv_pool = ctx.enter_context(tc.tile_pool(name="kv", bufs=2))
    q_pool = ctx.enter_context(tc.tile_pool(name="q", bufs=2))
    pt_pool = ctx.enter_context(tc.tile_pool(name="pt", bufs=5))
    sacc_pool = ctx.enter_context(tc.tile_pool(name="sacc", bufs=2))
    osb_pool = ctx.enter_context(tc.tile_pool(name="osb", bufs=3))
    bcs_pool = ctx.enter_context(tc.tile_pool(name="bcs", bufs=3))
    srow_pool = ctx.enter_context(tc.tile_pool(name="srow", bufs=3))
    srec_pool = ctx.enter_context(tc.tile_pool(name="srec", bufs=3))
    st_pool = ctx.enter_context(tc.tile_pool(name="st", bufs=2, space="PSUM"))
    oa_pool = ctx.enter_context(tc.tile_pool(name="oa", bufs=1, space="PSUM"))
    ps_sum = ctx.enter_context(tc.tile_pool(name="ps_sum", bufs=1, space="PSUM"))

    # Constants: tri[k, q] = 1.0 where q >= k (allowed), 0.0 where q < k.
    trif = singles.tile([128, 128], F32)
    make_upper_triangular(nc, trif[:], val=1.0, diag=True)
    tri = singles.tile([128, 128], BF16)
    nc.scalar.copy(out=tri[:], in_=trif[:])
    onesc = singles.tile([128, 1], BF16)   # ones column (sum-over-k lhsT)
    nc.vector.memset(onesc[:], 1.0)

    # One sum bank for the whole program; rows 0/32 and 64/96 alternate by
    # global chunk parity (subtile deps keep the parities independent).
    sum_ps = ps_sum.tile([128, 512], F32)

    kv_tiles = {}

    def ensure_pair(pair):
        if pair in kv_tiles or pair >= PAIRS_PER_CORE:
            return
        kt_sb = kv_pool.tile([D, S], BF16, tag="kt", name=f"kt_{pair}")
        nc.sync.dma_start(out=kt_sb[:], in_=kt[pair])
        v_sb = kv_pool.tile([128, NKT * D], BF16, tag="v", name=f"v_{pair}")
        nc.sync.dma_start(out=v_sb[:], in_=v[pair])
        kv_tiles[pair] = (kt_sb, v_sb)

    q_tiles = {}

    def ensure_head(head):
        if head in q_tiles or head >= HEADS_PER_CORE:
            return
        q_sb = q_pool.tile([D, S], BF16, tag="q", name=f"q_{head}")
        nc.sync.dma_start(out=q_sb[:], in_=qt[head])
        q_tiles[head] = q_sb

    # Flat schedule: (head, chunk, kti)
    sched = []
    for head in range(HEADS_PER_CORE):
        for c in range(NCH):
            for kti in range(8 * c + 8):
                sched.append((head, c, kti))

    # Per-(head,chunk) live state filled in while emitting
    oacc = {}      # (head, c) -> psum accumulator tile
    saccs = {}     # (head, c) -> (tile, base_off)
    osbs = {}      # (head, c) -> sbuf evacuation tile
    stages = {}    # sched index -> staging tile

    # Chunk-tail normalization pipeline, advanced one stage per iteration
    pending = []

    def advance_norm(now=-1, drain=False):
        # Process one stage of one (the oldest eligible) entry per call:
        # spreading the chunk-tail DVE/DMA burst across iterations keeps it
        # from delaying the next chunk's masks in the DVE queue.
        for ent in list(pending):
            if not drain and ent["born"] == now:
                continue
            head, c, st = ent["head"], ent["c"], ent["stage"]
            if st == 0:
                # pull the two sum rows out of PSUM (single partition,
                # 1024 wide) and DMA-reshape them to [128, 8]
                rb = ent["row_base"]
                srow = srow_pool.tile([1, CH], F32, tag="srow",
                                      name=f"srow_{head}_{c}")
                nc.vector.tensor_copy(srow[0:1, 0:512], sum_ps[rb:rb + 1, :])
                nc.vector.tensor_copy(
                    srow[0:1, 512:CH], sum_ps[rb + 32:rb + 33, :])
                srec = srec_pool.tile([128, NCH * 4], F32, tag="srec",
                                      name=f"srec_{head}_{c}")
                nc.sync.dma_start(out=srec[:], in_=srow[:])
                ent["srec"] = srec
            elif st == 1:
                srec2 = srec_pool.tile([128, NCH * 4], BF16, tag="srec2",
                                       name=f"srec2_{head}_{c}")
                with nc.allow_low_precision(reason="1/sum broadcast in bf16"):
                    nc.vector.reciprocal(out=srec2[:], in_=ent["srec"][:])
                nc.sync.dma_start(out=recd[head, c], in_=srec2[:])
            elif st == 2:
                bcs = bcs_pool.tile([128, CH], BF16, tag="bcs", name=f"bcs_{head}_{c}")
                nc.sync.dma_start(
                    out=bcs[:], in_=recd[head, c].partition_broadcast(128))
                ent["bcs"] = bcs
            elif st == 3:
                osb2 = osb_pool.tile([128, CH], BF16, tag="osb2",
                                     name=f"osb2_{head}_{c}")
                nc.vector.tensor_mul(osb2[:], ent["osb"][:], ent["bcs"][:])
                nc.sync.dma_start(
                    out=ot[head][:, CH * c:CH * (c + 1)], in_=osb2[:])
                pending.remove(ent)
            ent["stage"] = st + 1
            break

    def emit_qk(i):
        head, c, kti = sched[i]
        if c == 0 and kti == 0:
            pair = head // G
            ensure_pair(pair + 1)
            ensure_head(head + 1)
        kt_sb, _ = kv_tiles[head // G]
        q_sb = q_tiles[head]
        off = max(0, 128 * kti - CH * c)
        stage = st_pool.tile([128, CH], F32, tag="stage", name=f"st_{i}")
        for (s0, s1) in _segs(off):
            nc.tensor.matmul(
                out=stage[:, s0:s1],
                lhsT=kt_sb[:, 128 * kti:128 * (kti + 1)],
                rhs=q_sb[:, CH * c + s0:CH * c + s1],
                start=True, stop=True,
            )
        stages[i] = stage

    def emit_rest(i):
        head, c, kti = sched[i]
        _, v_sb = kv_tiles[head // G]
        off = max(0, 128 * kti - CH * c)
        last = kti == 8 * c + 7
        row_base = 64 * ((head * NCH + c) % 2)
        stage = stages.pop(i)

        # exp into SBUF (bf16); one wide ACTIVATE per k-tile
        p_kt = pt_pool.tile([128, CH], BF16, tag="pt", name=f"pt_{i}")
        nc.scalar.activation(
            p_kt[:, off:CH], stage[:, off:CH],
            mybir.ActivationFunctionType.Exp, scale=SCALE)

        # causal mask on the diagonal 128x128 block
        if 128 * kti >= CH * c:
            nc.vector.tensor_mul(
                p_kt[:, off:off + 128], p_kt[:, off:off + 128], tri[:])

        # out^T += V^T P^T
        if kti == 0:
            oacc[(head, c)] = oa_pool.tile(
                [128, CH], F32, tag="oacc", name=f"oa_{head}_{c}")
        # causality means cols [0,512) take their final AV write at
        # kti==8c+3; close that bank early so its evacuation overlaps the
        # remaining k-tiles instead of stalling the next chunk's matmuls
        oa = oacc[(head, c)]
        for (s0, s1) in _segs(off):
            bank_last = (kti == 8 * c + 3) if s1 <= 512 else last
            nc.tensor.matmul(
                out=oa[:, s0:s1],
                lhsT=v_sb[:, D * kti:D * (kti + 1)],
                rhs=p_kt[:, s0:s1],
                start=(kti == 0), stop=bank_last,
            )
        if kti == 8 * c + 3:
            osb = osb_pool.tile([128, CH], BF16, tag="osb",
                                name=f"osb_{head}_{c}")
            osbs[(head, c)] = osb
            nc.vector.tensor_copy(osb[:, 0:512], oa[:, 0:512])

        # denominators: a PE/DVE split keeping the PE (the bottleneck
        # engine) light: k-tiles {0,4,8,12} go to the PE as ones-column
        # matmuls, the rest accumulate element-wise on the DVE
        if kti % 4 == 0:
            for (s0, s1) in _segs(off):
                r, c0, c1 = _sum_rc(row_base, s0, s1)
                nc.tensor.matmul(
                    out=sum_ps[r:r + 1, c0:c1],
                    lhsT=onesc[:],
                    rhs=p_kt[:, s0:s1],
                    start=(kti == 0), stop=False,
                    tile_position=(0, r),
                )
        else:
            key = (head, c)
            if key not in saccs:
                sacc = sacc_pool.tile([128, CH], BF16, tag="sacc",
                                      name=f"sacc_{head}_{c}")
                nc.vector.tensor_copy(sacc[:, off:CH], p_kt[:, off:CH])
                saccs[key] = (sacc, off)
            else:
                sacc, _ = saccs[key]
                nc.vector.tensor_add(
                    sacc[:, off:CH], sacc[:, off:CH], p_kt[:, off:CH])

        if last:
            # fold the DVE partial sums into the PSUM sum rows
            sacc, base = saccs.pop((head, c))
            for (s0, s1) in _segs(base):
                r, c0, c1 = _sum_rc(row_base, s0, s1)
                nc.tensor.matmul(
                    out=sum_ps[r:r + 1, c0:c1],
                    lhsT=onesc[:],
                    rhs=sacc[:, s0:s1],
                    start=False, stop=True,
                    tile_position=(0, r),
                )
            # evacuate the hi half of the accumulator (lo went at 8c+3)
            oa = oacc.pop((head, c))
            osb = osbs.pop((head, c))
            nc.vector.tensor_copy(osb[:, 512:CH], oa[:, 512:CH])
            pending.append(dict(head=head, c=c, stage=0, osb=osb,
                                row_base=row_base, born=i))

    # QK runs two iterations ahead of the rest: QK(i+2) only WAR-depends
    # on exp(i) (staging rotation), so it sits in the PE queue BEFORE
    # AV(i) and the exp stream never transitively waits on the mask/AV
    # path -- the PE and ACT streams decouple.
    ensure_pair(0)
    ensure_head(0)
    emit_qk(0)
    emit_qk(1)
    for i in range(len(sched)):
        if i + 2 < len(sched):
            emit_qk(i + 2)
        emit_rest(i)
        advance_norm(now=i)
    while pending:
        advance_norm(drain=True)


_CACHED_NC = None


def build_program():
    global _CACHED_NC
    if _CACHED_NC is not None:
        return _CACHED_NC
    nc = bacc.Bacc("TRN2", target_bir_lowering=False, debug=False,
                   num_devices=NCORES)
    qt = nc.dram_tensor("qt", [HEADS_PER_CORE, D, S], BF16,
                        kind="ExternalInput").ap()
    kt = nc.dram_tensor("kt", [PAIRS_PER_CORE, D, S], BF16,
                        kind="ExternalInput").ap()
    v = nc.dram_tensor("v", [PAIRS_PER_CORE, 128, NKT * D], BF16,
                       kind="ExternalInput").ap()
    recd = nc.dram_tensor("recd", [HEADS_PER_CORE, NCH, CH], BF16,
                          kind="Internal").ap()
    ot = nc.dram_tensor("ot", [HEADS_PER_CORE, D, S], BF16,
                        kind="ExternalOutput").ap()
    with tile.TileContext(nc) as tc:
        emit_core_program(tc, qt, kt, v, recd, ot)
    nc.compile()
    _CACHED_NC = nc
    return nc


def shard_inputs(query, key, value):
    """Full inputs -> list of 8 per-core in_maps (host relayout + bf16)."""
    query = np.asarray(query, dtype=np.float32)
    key = np.asarray(key, dtype=np.float32)
    value = np.asarray(value, dtype=np.float32)

    # Q: [S,B,HQ,D] -> [B*HKV, G, D, S]
    qtall = np.ascontiguousarray(
        query.reshape(S, B, HKV, G, D).transpose(1, 2, 3, 4, 0)
    ).reshape(NPAIRS, G, D, S).astype(NP_BF16)
    # K: [S,B,HKV,D] -> [B*HKV, D, S]
    ktall = np.ascontiguousarray(
        key.transpose(1, 2, 3, 0)).reshape(NPAIRS, D, S).astype(NP_BF16)
    # V: [S,B,HKV,D] -> [B*HKV, k_local=128, NKT*D]
    vall = np.ascontiguousarray(
        value.reshape(NKT, 128, B, HKV, D).transpose(2, 3, 1, 0, 4)
    ).reshape(NPAIRS, 128, NKT * D).astype(NP_BF16)

    in_maps = []
    for c in range(NCORES):
        p0 = PAIRS_PER_CORE * c
        p1 = p0 + PAIRS_PER_CORE
        in_maps.append({
            "qt": np.ascontiguousarray(qtall[p0:p1].reshape(HEADS_PER_CORE, D, S)),
            "kt": np.ascontiguousarray(ktall[p0:p1]),
            "v": np.ascontiguousarray(vall[p0:p1]),
        })
    return in_maps


def unshard_output(results):
    """8 per-core {'ot': [8, D, S]} -> full [S, B, HQ, D]."""
    ot = np.stack([np.asarray(r["ot"], dtype=np.float32) for r in results])
    ot = ot.reshape(B, HKV, G, D, S)                   # pairs major -> b, hkv
    out = np.ascontiguousarray(ot.transpose(4, 0, 1, 2, 3))  # [S,B,HKV,G,D]
    return out.reshape(S, B, HQ, D)


def kernel(query, key, value, _trace=False, _return_bkr=False):
    nc = build_program()
    in_maps = shard_inputs(query, key, value)
    bkr = bass_utils.run_bass_kernel_spmd(
        nc, in_maps, core_ids=list(range(NCORES)), trace=_trace)
    out = unshard_output(bkr.results)
    if _return_bkr:
        return out, bkr
    return out


if __name__ == "__main__":
    q = np.random.randn(S, B, HQ, D).astype(np.float32)
    k = np.random.randn(S, B, HKV, D).astype(np.float32)
    vv = np.random.randn(S, B, HKV, D).astype(np.float32)
    o = kernel(q, k, vv)
    print("out", o.shape, o.dtype, float(np.abs(o).max()))

